# revision 67
# baseline (speedup 1.0000x reference)
"""Trainium2 Bass kernel for one dense transformer block (B=4, S=2048, D=768, H=12).

Sharding: 8 cores = 4 batches x 2 interleaved sequence halves, no collectives.
Core (b, h) owns 8 of the 16 128-token blocks of batch b. Within each
512-token context chunk the 4 blocks are host-permuted so the core's own 2
blocks always sit at columns 128:384 — Q-projection reads a fixed slice of
the LN'd context chunk, so LayerNorm over the context is done exactly once.
Key/mask order follows the same permutation (host builds masks to match).

Feature-major everywhere ([features-on-partitions, tokens-free]); no on-device
transposes. LN gains and the V bias are folded into adjacent weights on the
host. Softmax denominators ride as a ones-column appended to V; no
max-subtraction (scores are tiny after the 1/sqrt(768) scale).

vs v0: chunk-contiguous DRAM layouts for all streamed tensors, weight
streaming for fc1/fc2 (per-tile double-buffered DMA instead of monolithic
loads), QKV weights split and DMA'd in use order, LN stats matmuls col-packed
(sum and sumsq run concurrently in different PE column groups), score matmuls
for the even/odd head of a pair issued back-to-back (disjoint PE row groups
-> 2x concurrency), PSUM->SBUF copies with bias moved to the ACT engine, and
softmax normalization folded into the attention head-pair loop.

vs v1: fc1 runs fp8 DoubleRow (weights and LN2'd activations in e4m3, gelu
rescales on evacuation; adds ~1.2e-2 rel err, well under the 2e-2 gate and
halves fc1 PE cycles); the first half of w2 streams in on idle DMA lanes
from kernel start so fc2(0) starts immediately after attention and hides
LN2(g1)'s serial chain (fc2 is j-outer across 6 persistent PSUM
accumulators, enabled by releasing the attention PSUM pools early) -- this
removed a ~40us all-engine valley plus the HAM cold-restart it caused; V
evacuation alternates DVE/ACT to unload the projection-phase DVE hotspot.

vs v2: attn@V runs fp8 DoubleRow over key-tile pairs (one DR matmul per head
per pair; vaug holds 16*v in e4m3 padded to 96 so every DR lhsT slice base
is 16B-aligned; pt in fp8; pair-sliced masks halve the DVE mask ops). The
cps evacuations scale by 1/16 — vaug's x16 means raw numerators (~400)
would overflow fp8's 240 max. w2's first half streams during attention,
emitted after the projection-phase DMAs so it can't delay the LN chunk
prefetches at startup. Measured dead ends: fp8 kT/qT score matmuls are
slower than bf16 (non-DR f8 LDWEIGHTS), and moving exp work to DVE loses
to the critical-path schedule.
"""

import math

import numpy as np
import ml_dtypes

import concourse.bass as bass
import concourse.mybir as mybir
import concourse.tile as tile
from concourse.bass_utils import run_bass_kernel_spmd
from concourse.vector_clock import ScopedClock

AF = mybir.ActivationFunctionType
ALU = mybir.AluOpType
BF16 = mybir.dt.bfloat16
F32 = mybir.dt.float32
F8 = mybir.dt.float8e4
PM_DR = mybir.MatmulPerfMode.DoubleRow
W8SCALE = 16.0

B, S, D, H = 4, 2048, 768, 12
HD = D // H          # 64
EPS = 1e-5
P = 128
KSUB = D // P        # 6
NB = S // P          # 16 key-block positions per batch
SQ = S // 2          # 1024 own queries per core
NCH = S // 512       # 4 context chunks
N_CORES = 8
NREP = 1
SCALE = 1.0 / math.sqrt(D)


def own_blocks(h: int) -> list[int]:
    out = []
    for j in range(4):
        out += [4 * j, 4 * j + 3] if h == 0 else [4 * j + 1, 4 * j + 2]
    return out


def block_perm(h: int) -> list[int]:
    """Block order inside each 4-block chunk; own blocks land at positions 1,2."""
    return [1, 0, 3, 2] if h == 0 else [0, 1, 2, 3]


class SplitDrainTileContext(tile.TileContext):
    """walrus here rejects a Drain carrying >1 sync-wait; split the kernel-tail
    drain into one Drain per semaphore wait."""

    def _drain_and_barrier(self, tick_clock, wait_clock):
        nc = self.nc
        drain_inst = nc.sync.drain()
        wait_clock.add_sem_waits(
            drain_inst.ins, ScopedClock({None: tick_clock.global_clock})
        )
        nc.all_engine_barrier()
        assert self.sems is not None
        popped = nc._tile_sem_poison_stack.pop()
        assert popped is self._sem_poison
        nc.clear_and_free_semaphores(list(self.sems.allocated().values()))
        nc.all_engine_barrier()
        self._split_multi_waits(nc)

    @staticmethod
    def _split_multi_waits(nc):
        k = 0
        for bb in nc.main_func.blocks:
            out = []
            for ins in bb.instructions:
                si = ins.sync_info
                if si is not None and si.on_wait and len(si.on_wait) > 1:
                    waits = list(si.on_wait)
                    si.on_wait = [waits[-1]]
                    for w in waits[:-1]:
                        k += 1
                        out.append(
                            mybir.InstNoOp(
                                name=f"{ins.name}_sw{k}",
                                engine=ins.engine,
                                ins=[],
                                outs=[],
                                sync_info=mybir.SyncInfo(on_wait=[w], on_update=[]),
                            )
                        )
                out.append(ins)
            try:
                bb.instructions[:] = out
            except TypeError:
                bb.set_instructions(out)


def _ln_chunk(nc, lnp, lnrows, psum_stat, psum_bc, xb, xnch, ones, onesr, eps_row,
              stat_tag="ln_ps", bc_tag="bc"):
    """One 512-token LayerNorm chunk, feature-major.

    Stats via ones-matmuls: sum(x) accumulates at PSUM partition 0, sum(x^2)
    at partition 32 — issued interleaved so the two accumulation chains run
    in different PE column groups concurrently. Row math then happens at
    partition 32 (where sum(x^2) landed); mu reaches partition 32 through
    its broadcast matmul. rstd = exp(-0.5*ln(var+eps)) avoids the slow
    iterative divide.
    """
    ps = psum_stat.tile([P, 512], F32, tag=stat_tag)
    for j in range(KSUB):
        sq = lnp.tile([P, 512], BF16, tag="ln_sq")
        nc.scalar.activation(sq[:], xb[:, j, :], AF.Square)
        nc.tensor.matmul(
            ps[0:1, :], ones[:], xb[:, j, :], start=(j == 0), stop=(j == KSUB - 1)
        )
        nc.tensor.matmul(
            ps[32:33, :], ones[:], sq[:], start=(j == 0), stop=(j == KSUB - 1)
        )
    mu = lnrows.tile([1, 512], BF16, tag="ln_mu")
    nc.vector.tensor_scalar_mul(mu[:], ps[0:1, :], 1.0 / D)
    mu_b = psum_bc.tile([P, 512], F32, tag=bc_tag)
    nc.tensor.matmul(mu_b[:], onesr[0:1, :], mu[:], start=True, stop=True)
    # var/rstd at partition 32 (where sum(x^2) landed)
    rows32 = lnrows.tile([33, 512], F32, tag="ln_r32")
    m2 = rows32[32:33, :]
    nc.scalar.activation(m2, mu_b[32:33, :], AF.Square)
    var = m2
    nc.vector.scalar_tensor_tensor(
        out=var, in0=ps[32:33, :], scalar=1.0 / D, in1=m2,
        op0=ALU.mult, op1=ALU.subtract,
    )
    lnv_t = lnrows.tile([33, 512], F32, tag="ln_lnv")
    lnv = lnv_t[32:33, :]
    nc.scalar.activation(lnv, var, AF.Ln, bias=eps_row[32:33, :])
    rn = lnrows.tile([33, 1024], BF16, tag="ln_rn")
    rstd = rn[32:33, 0:512]
    nc.scalar.activation(rstd, lnv, AF.Exp, scale=-0.5)
    negmur = rn[32:33, 512:1024]
    nc.vector.scalar_tensor_tensor(
        out=negmur, in0=mu_b[32:33, :], scalar=-1.0, in1=rstd,
        op0=ALU.mult, op1=ALU.mult,
    )
    rstd_b = psum_bc.tile([P, 512], F32, tag=bc_tag)
    nc.tensor.matmul(rstd_b[:], onesr[32:33, :], rstd, start=True, stop=True)
    negmur_b = psum_bc.tile([P, 512], F32, tag=bc_tag)
    nc.tensor.matmul(negmur_b[:], onesr[32:33, :], negmur, start=True, stop=True)
    nc.vector.tensor_tensor(
        xnch[:], xb[:], rstd_b[:, None, :].to_broadcast([P, KSUB, 512]), ALU.mult
    )
    nc.vector.tensor_tensor(
        xnch[:], xnch[:], negmur_b[:, None, :].to_broadcast([P, KSUB, 512]), ALU.add
    )


def build_nc():
    nc = bass.Bass("TRN2", num_devices=N_CORES)
    xc = nc.declare_dram_parameter("xc", [NCH, P, KSUB, 512], F8, isOutput=False)
    xq = nc.declare_dram_parameter("xq", [2, P, KSUB, 512], BF16, isOutput=False)
    wkd = nc.declare_dram_parameter("wk", [P, 3, 2, D], F8, isOutput=False)
    wvd = nc.declare_dram_parameter("wv", [P, 3, 2, D], F8, isOutput=False)
    wqd = nc.declare_dram_parameter("wq", [P, 3, 2, D], F8, isOutput=False)
    wod = nc.declare_dram_parameter("wo", [P, 3, 2, D], F8, isOutput=False)
    w1d = nc.declare_dram_parameter("w1", [P, 4 * KSUB, 3, 2, P], F8, isOutput=False)
    w2d = nc.declare_dram_parameter("w2", [P, 4 * KSUB, D], BF16, isOutput=False)
    bqkd = nc.declare_dram_parameter("bqk", [P, 12], F32, isOutput=False)
    bod = nc.declare_dram_parameter("bo", [P, KSUB], F32, isOutput=False)
    b1d = nc.declare_dram_parameter("b1", [P, 4 * KSUB], F32, isOutput=False)
    b2d = nc.declare_dram_parameter("b2", [P, KSUB], F32, isOutput=False)
    maskd = nc.declare_dram_parameter("mask", [P, NB, P], F8, isOutput=False)
    sel2d = nc.declare_dram_parameter("sel2", [2, P], BF16, isOutput=False)
    yT = nc.declare_dram_parameter("yT", [P, KSUB, SQ], F32, isOutput=True)

    with SplitDrainTileContext(nc) as tc:
        for it in range(NREP):
            # LEFT stack: long-lived pools
            persist = tc.alloc_tile_pool(name=f"persist{it}", bufs=1, side="left")
            lnp = tc.alloc_tile_pool(name=f"lnp{it}", bufs=2, side="left")
            lnrows = tc.alloc_tile_pool(name=f"lnrows{it}", bufs=1, side="left")
            xbp = tc.alloc_tile_pool(name=f"xbp{it}", bufs=3, side="left")
            w2p = tc.alloc_tile_pool(name=f"w2p{it}", bufs=1, side="left")
            stage = tc.alloc_tile_pool(name=f"stage{it}", bufs=2, side="left")
            h1pool = tc.alloc_tile_pool(name=f"h1{it}", bufs=1, side="left")
            ptpool = tc.alloc_tile_pool(name=f"pt{it}", bufs=3, side="left")
            w1s = tc.alloc_tile_pool(name=f"w1s{it}", bufs=3, side="left")
            wop = tc.alloc_tile_pool(name=f"wop{it}", bufs=1, side="left")
            xnp = tc.alloc_tile_pool(name=f"xnp{it}", bufs=3, side="left")
            # RIGHT stack: phase-scoped big pools
            pool_attn = tc.alloc_tile_pool(name=f"attn{it}", bufs=1, side="right")
            pool_w = tc.alloc_tile_pool(name=f"wkvq{it}", bufs=1, side="right")
            psum_mm = tc.alloc_tile_pool(name=f"psum_mm{it}", bufs=2, space="PSUM")
            psum_stat = tc.alloc_tile_pool(name=f"psum_stat{it}", bufs=1, space="PSUM")
            psum_bc = tc.alloc_tile_pool(name=f"psum_bc{it}", bufs=3, space="PSUM")

            # ---- front DMAs in use order: first x chunk, then K weights ----
            xch0 = xbp.tile([P, KSUB, 512], F8, tag="ln_xb")
            nc.sync.dma_start(out=xch0[:], in_=xc[0])
            wk_s = pool_w.tile([P, 3, 2, D], F8)
            nc.sync.dma_start(out=wk_s[:], in_=wkd[:])

            ones = persist.tile([P, 1], BF16)
            nc.vector.memset(ones[:], 1.0)
            eps_row = persist.tile([33, 1], F32)
            nc.vector.memset(eps_row[:], EPS)
            onesr = persist.tile([33, P], BF16)
            nc.vector.memset(onesr[:], 1.0)
            sel2 = persist.tile([2, P], BF16)
            nc.sync.dma_start(out=sel2[:], in_=sel2d[:])
            bqk_s = persist.tile([P, 12], F32)
            nc.sync.dma_start(out=bqk_s[:], in_=bqkd[:])
            bo_s = persist.tile([P, KSUB], F32)
            nc.sync.dma_start(out=bo_s[:], in_=bod[:])
            b1_s = persist.tile([P, 4 * KSUB], F32)
            nc.sync.dma_start(out=b1_s[:], in_=b1d[:])
            b2_s = persist.tile([P, KSUB], F32)
            nc.sync.dma_start(out=b2_s[:], in_=b2d[:])
            wv_s = pool_w.tile([P, 3, 2, D], F8)
            nc.sync.dma_start(out=wv_s[:], in_=wvd[:])
            wq_s = pool_w.tile([P, 3, 2, D], F8)
            nc.sync.dma_start(out=wq_s[:], in_=wqd[:])
            # first half of w2 streams in during attention (emitted after the
            # projection-phase DMAs so it can't delay the LN chunk
            # prefetches at startup); fc2(0) still never waits on it
            w2_lo = w2p.tile([P, 2 * KSUB, D], BF16)

            # PE warmup: back-to-back matmuls at kernel start overlap the
            # first input DMAs so the HAM clock-gate reaches 2.4 GHz early.
            warm_src = persist.tile([1, 512], BF16)
            nc.vector.memset(warm_src[:], 1.0)
            for wi in range(6):
                wp = psum_mm.tile([P, 512], F32, tag="mm")
                nc.tensor.matmul(wp[:], onesr[0:1, :], warm_src[:], start=True, stop=True)

            mask_s = pool_attn.tile([P, NB, P], F8)
            kT = pool_attn.tile([P, KSUB, S], BF16)
            # vaug holds 16*v in fp8, padded to 80 so every DR lhsT slice
            # BASE (kt*H*80 + head*80) and stride is 16B-aligned — dual-fp8
            # LDWEIGHTS silently corrupts on misaligned bases (walrus only
            # checks strides). Ones-column is 16 so softmax numerator and
            # denominator share the scale and sel2's x16 trick is unchanged.
            vaug = pool_attn.tile([P, NB, H, 96], F8)
            qT = pool_attn.tile([P, KSUB, SQ], BF16)
            nc.vector.memset(vaug[:, :, :, HD : HD + 1], W8SCALE)

            # ======== LN1 fused with K/V-proj (full context) + Q-proj (own cols) ====
            # software-pipelined: chunk c+1's LayerNorm runs between chunk
            # c's K-proj and Q/V-proj so its serial stats->apply chain hides
            # under chunk c's projection matmuls
            # two-deep LN pipeline: chunk c+2's LayerNorm chain hides under
            # two chunks of projection matmuls
            xch1 = xbp.tile([P, KSUB, 512], F8, tag="ln_xb", name="xch1")
            nc.sync.dma_start(out=xch1[:], in_=xc[1])
            lnq = []
            xn0 = xnp.tile([P, KSUB, 512], F8, tag="xnch", name="xnch0")
            _ln_chunk(nc, lnp, lnrows, psum_stat, psum_bc, xch0, xn0,
                      ones, onesr, eps_row)
            lnq.append(xn0)
            xn1 = xnp.tile([P, KSUB, 512], F8, tag="xnch", name="xnch1")
            _ln_chunk(nc, lnp, lnrows, psum_stat, psum_bc, xch1, xn1,
                      ones, onesr, eps_row)
            lnq.append(xn1)
            for c in range(NCH):
                xnch = lnq[c]
                if c + 2 < NCH:
                    xch = xbp.tile([P, KSUB, 512], F8, tag="ln_xb",
                                   name=f"xch{c + 2}")
                    nc.sync.dma_start(out=xch[:], in_=xc[c + 2])
                # K-proj: kT[:, m, 512c:512c+512]
                for m in range(KSUB):
                    ps = psum_mm.tile([P, 512], F32, tag="mm", name=f"kps{c}_{m}")
                    for jj in range(3):
                        nc.tensor.matmul(
                            ps[:],
                            wk_s[:, jj, :, P * m : P * (m + 1)],
                            xnch[:, 2 * jj : 2 * jj + 2, :],
                            start=(jj == 0),
                            stop=(jj == 2),
                            perf_mode=PM_DR,
                        )
                    nc.scalar.activation(
                        kT[:, m, 512 * c : 512 * c + 512], ps[:], AF.Identity,
                        bias=bqk_s[:, 6 + m : 7 + m], scale=1.0 / W8SCALE,
                    )
                if c + 2 < NCH:
                    xnch_next = xnp.tile([P, KSUB, 512], F8, tag="xnch",
                                         name=f"xnch{c + 2}")
                    _ln_chunk(nc, lnp, lnrows, psum_stat, psum_bc, xch,
                              xnch_next, ones, onesr, eps_row)
                    lnq.append(xnch_next)
                # Q-proj for own tokens (fixed columns 128:384 of every chunk)
                for m in range(KSUB):
                    ps = psum_mm.tile([P, 256], F32, tag="mmq", name=f"qps{c}_{m}")
                    for jj in range(3):
                        nc.tensor.matmul(
                            ps[:],
                            wq_s[:, jj, :, P * m : P * (m + 1)],
                            xnch[:, 2 * jj : 2 * jj + 2, 128:384],
                            start=(jj == 0),
                            stop=(jj == 2),
                            perf_mode=PM_DR,
                        )
                    nc.scalar.activation(
                        qT[:, m, 256 * c : 256 * (c + 1)], ps[:], AF.Identity,
                        bias=bqk_s[:, m : m + 1], scale=1.0 / W8SCALE,
                    )
                # V-proj: token-major 128-blocks, per-head stride 65
                for t in range(4):
                    tt = 4 * c + t
                    for half in range(2):
                        ps = psum_mm.tile([P, 384], F32, tag="mm")
                        for jj in range(3):
                            nc.tensor.matmul(
                                ps[:],
                                xnch[:, 2 * jj : 2 * jj + 2, P * t : P * (t + 1)],
                                wv_s[:, jj, :, 384 * half : 384 * (half + 1)],
                                start=(jj == 0),
                                stop=(jj == 2),
                                perf_mode=PM_DR,
                            )
                        if half == 0:
                            nc.vector.tensor_copy(
                                vaug[:, tt, 6 * half : 6 * (half + 1), 0:HD],
                                ps[:].rearrange("p (h d) -> p h d", d=HD),
                            )
                        else:
                            # split V evacuation across DVE and ACT: DVE is
                            # the busiest engine during the projection phase
                            nc.scalar.activation(
                                vaug[:, tt, 6 * half : 6 * (half + 1), 0:HD],
                                ps[:].rearrange("p (h d) -> p h d", d=HD),
                                AF.Identity,
                            )

            # queued after the context-chunk DMAs: needed from attention on
            nc.sync.dma_start(out=mask_s[:], in_=maskd[:])
            wo_s = wop.tile([P, 3, 2, D], F8)
            nc.sync.dma_start(out=wo_s[:], in_=wod[:])
            for j in range(0, 2 * KSUB, 3):
                nc.sync.dma_start(out=w2_lo[:, j : j + 3], in_=w2d[:, j : j + 3])

            pool_w.release()
            psum_bc.release()
            psum_stat.release()
            psum_mm.release()
            xnp.release()

            # ======== attention (per query half) interleaved with MLP ========
            # PSUM plan (exactly 8 banks): ctx 2 + scores 4 + all-purpose
            # [P,512] ring 2 (out-proj / fc1 / fc2 / LN2 stats+broadcasts /
            # softmax-denominator broadcasts all share the ring).
            ctxp = tc.alloc_tile_pool(name=f"ctxp{it}", bufs=1, side="left")
            g0p = tc.alloc_tile_pool(name=f"g0p{it}", bufs=1, side="left")
            psum_mm3 = tc.alloc_tile_pool(name=f"psum_mm3{it}", bufs=2, space="PSUM")
            psum_ctx = tc.alloc_tile_pool(name=f"psum_ctx{it}", bufs=2, space="PSUM")
            # two tags (even/odd head score tiles) x 1 buf x 2 banks = 4 banks
            psum_mm2 = tc.alloc_tile_pool(name=f"psum_mm2{it}", bufs=1, space="PSUM")

            ctx = ctxp.tile([P, KSUB, SQ], F8)
            h1 = h1pool.tile([P, KSUB, SQ], F32)

            def attn_half(g):
                n_kt = 8 * g + 8
                gsl = slice(512 * g, 512 * (g + 1))
                for p in range(KSUB):
                    k0 = kT[0:HD, p, :]
                    k1 = kT[HD:P, p, :]
                    q0 = qT[0:HD, p, 512 * g : 512 * (g + 1)]
                    q1 = qT[HD:P, p, 512 * g : 512 * (g + 1)]
                    cps0 = psum_ctx.tile([HD + 1, 512], F32, tag="ctx", name=f"cps0_{g}_{p}")
                    cps1 = psum_ctx.tile([HD + 1, 512], F32, tag="ctx", name=f"cps1_{g}_{p}")
                    npair = n_kt // 2
                    for u in range(npair):
                        # kt pair (2u, 2u+1): same qoff (kt//2 == u), so the
                        # pair shares one fp8 DoubleRow attn@V matmul per head
                        kta = 2 * u
                        qoff = (max(4 * g, u) - 4 * g) * 128
                        width = 512 - qoff
                        sps0 = psum_mm2.tile([P, 2, 512], F32, tag="mm2", name=f"sps0_{g}_{p}_{u}")
                        sps1 = psum_mm2.tile([P, 2, 512], F32, tag="mm2b", name=f"sps1_{g}_{p}_{u}")
                        pt0 = ptpool.tile([P, 2, 512], F8, tag="pt0", name=f"pt0_{g}_{p}_{u}")
                        pt1 = ptpool.tile([P, 2, 512], F8, tag="pt1", name=f"pt1_{g}_{p}_{u}")
                        for o in range(2):
                            # even/odd head issued back-to-back into different
                            # banks: disjoint PE row groups run concurrently
                            nc.tensor.matmul(
                                sps0[:, o, :width], k0[:, P * (kta + o) : P * (kta + o + 1)],
                                q0[:, qoff:512], start=True, stop=True,
                            )
                            nc.tensor.matmul(
                                sps1[:, o, :width], k1[:, P * (kta + o) : P * (kta + o + 1)],
                                q1[:, qoff:512], start=True, stop=True,
                            )
                        nc.scalar.activation(
                            pt0[:, :, :width], sps0[:, :, :width], AF.Exp,
                            scale=SCALE,
                        )
                        nc.scalar.activation(
                            pt1[:, :, :width], sps1[:, :, :width], AF.Exp,
                            scale=SCALE,
                        )
                        if u >= 4 * g:
                            nc.vector.tensor_mul(
                                pt0[:, :, 0:P], pt0[:, :, 0:P],
                                mask_s[:, kta : kta + 2, :],
                            )
                            nc.vector.tensor_mul(
                                pt1[:, :, 0:P], pt1[:, :, 0:P],
                                mask_s[:, kta : kta + 2, :],
                            )
                        nc.tensor.matmul(
                            cps0[:, qoff:512], vaug[:, kta : kta + 2, 2 * p, 0 : HD + 1],
                            pt0[:, :, :width],
                            start=(u == 0), stop=(u == npair - 1),
                            perf_mode=PM_DR,
                        )
                        nc.tensor.matmul(
                            cps1[:, qoff:512], vaug[:, kta : kta + 2, 2 * p + 1, 0 : HD + 1],
                            pt1[:, :, :width],
                            start=(u == 0), stop=(u == npair - 1),
                            perf_mode=PM_DR,
                        )
                    # evacuate: even head direct to partitions 0:64, odd head
                    # staged + DMA'd to partitions 64:128; l rows to rpair
                    nc.vector.tensor_scalar_mul(ctx[0:HD, p, gsl], cps0[0:HD, :], 1.0 / W8SCALE)
                    lt = stage.tile([HD + 1, 512], BF16, tag="lstage", name=f"lt{g}_{p}")
                    nc.vector.tensor_scalar_mul(lt[HD : HD + 1, :], cps0[HD : HD + 1, :], 1.0 / W8SCALE)
                    st8 = stage.tile([HD, 512], F8, tag="cstage", name=f"st{g}_{p}")
                    nc.vector.tensor_scalar_mul(st8[:], cps1[0:HD, :], 1.0 / W8SCALE)
                    lt2 = stage.tile([HD + 1, 512], BF16, tag="lstage2", name=f"lt2{g}_{p}")
                    nc.vector.tensor_scalar_mul(lt2[HD : HD + 1, :], cps1[HD : HD + 1, :], 1.0 / W8SCALE)
                    rpair = stage.tile([2, 512], BF16, tag="rpair", name=f"rp{g}_{p}")
                    nc.sync.dma_start(out=rpair[0:1, :], in_=lt[HD : HD + 1, :])
                    nc.sync.dma_start(out=rpair[1:2, :], in_=lt2[HD : HD + 1, :])
                    nc.sync.dma_start(out=ctx[HD:P, p, gsl], in_=st8[:])
                    rr = stage.tile([2, 512], BF16, tag="rr", name=f"rr{g}_{p}")
                    with nc.allow_low_precision(reason="softmax denoms fit bf16"):
                        nc.vector.reciprocal(rr[:], rpair[:])
                    rb = psum_mm3.tile([P, 512], F32, tag="mm", name=f"rb{g}_{p}")
                    nc.tensor.matmul(rb[:], sel2[:], rr[:], start=True, stop=True)
                    nc.vector.tensor_mul(ctx[:, p, gsl], ctx[:, p, gsl], rb[:])

            def outproj_half(g):
                gsl = slice(512 * g, 512 * (g + 1))
                xres = xbp.tile([P, KSUB, 512], BF16, tag="ln_xb", name=f"xres{g}")
                nc.sync.dma_start(out=xres[:], in_=xq[g])
                for m in range(KSUB):
                    ps = psum_mm3.tile([P, 512], F32, tag="mm", name=f"ops{g}_{m}")
                    for jj in range(3):
                        nc.tensor.matmul(
                            ps[:],
                            wo_s[:, jj, :, P * m : P * (m + 1)],
                            ctx[:, 2 * jj : 2 * jj + 2, gsl],
                            start=(jj == 0),
                            stop=(jj == 2),
                            perf_mode=PM_DR,
                        )
                    otmp = stage.tile([P, 512], BF16, tag="otmp", name=f"ot{g}_{m}")
                    nc.scalar.activation(
                        otmp[:], ps[:], AF.Identity, bias=bo_s[:, m : m + 1],
                        scale=1.0 / (W8SCALE * W8SCALE),
                    )
                    nc.vector.tensor_tensor(
                        h1[:, m, gsl], otmp[:], xres[:, m, :], ALU.add
                    )

            def ln2_half(g, xn2g):
                gsl = slice(512 * g, 512 * (g + 1))
                hb = xbp.tile([P, KSUB, 512], BF16, tag="ln_xb", name=f"hb{g}")
                nc.vector.tensor_copy(hb[:], h1[:, :, gsl])
                _ln_chunk(nc, lnp, lnrows, psum_mm3, psum_mm3, hb, xn2g[:],
                          ones, onesr, eps_row, stat_tag="mm", bc_tag="mm")

            def fc1_half(g, xn2g, gug):
                for m in range(4 * KSUB):
                    w1t = w1s.tile([P, 3, 2, P], F8, tag="w1t", name=f"w1t{g}_{m}")
                    nc.sync.dma_start(out=w1t[:], in_=w1d[:, m])
                    ps = psum_mm3.tile([P, 512], F32, tag="mm", name=f"f1ps{g}_{m}")
                    for jj in range(3):
                        nc.tensor.matmul(
                            ps[:],
                            w1t[:, jj, :, :],
                            xn2g[:, 2 * jj : 2 * jj + 2, :],
                            start=(jj == 0),
                            stop=(jj == 2),
                            perf_mode=PM_DR,
                        )
                    nc.scalar.activation(
                        gug[:, m, :], ps[:], AF.Gelu, bias=b1_s[:, m : m + 1],
                        scale=1.0 / W8SCALE,
                    )

            def _w2t(j):
                return w2_lo[:, j] if j < 2 * KSUB else w2_hi[:, j - 2 * KSUB]

            def fc2_half(g, gug, w2_hi, psum_fc2):
                gsl = slice(512 * g, 512 * (g + 1))
                if True:
                    # j-outer with 6 persistent PSUM accumulators: the first
                    # 12 j-tiles (w2_lo) are resident from the early DMA, so
                    # fc2 starts while w2_hi is still in flight
                    pss = [
                        psum_fc2.tile([P, 512], F32, tag=f"fc2m{m}", name=f"f2ps{g}_{m}")
                        for m in range(KSUB)
                    ]
                    for j in range(4 * KSUB):
                        w2t = _w2t(j)
                        for m in range(KSUB):
                            nc.tensor.matmul(
                                pss[m][:],
                                w2t[:, P * m : P * (m + 1)],
                                gug[:, j, :],
                                start=(j == 0),
                                stop=(j == 4 * KSUB - 1),
                            )
                    for m in range(KSUB):
                        yt = stage.tile([P, 512], F32, tag="ystage", name=f"yt{g}_{m}")
                        nc.vector.scalar_tensor_tensor(
                            out=yt[:], in0=pss[m][:], scalar=b2_s[:, m : m + 1],
                            in1=h1[:, m, gsl], op0=ALU.add, op1=ALU.add,
                        )
                        nc.sync.dma_start(out=yT[:, m, gsl], in_=yt[:])
                else:
                    # w2 fully resident by now; m-outer drains each PSUM bank
                    # as soon as its chain ends so the tail evac+store DMA
                    # pipeline overlaps the remaining matmuls
                    for m in range(KSUB):
                        ps = psum_fc2.tile([P, 512], F32, tag=f"fc2m{m % 2}", name=f"f2ps{g}_{m}")
                        for j in range(4 * KSUB):
                            nc.tensor.matmul(
                                ps[:],
                                _w2t(j)[:, P * m : P * (m + 1)],
                                gug[:, j, :],
                                start=(j == 0),
                                stop=(j == 4 * KSUB - 1),
                            )
                        yt = stage.tile([P, 512], F32, tag="ystage", name=f"yt{g}_{m}")
                        nc.vector.scalar_tensor_tensor(
                            out=yt[:], in0=ps[:], scalar=b2_s[:, m : m + 1],
                            in1=h1[:, m, gsl], op0=ALU.add, op1=ALU.add,
                        )
                        nc.sync.dma_start(out=yT[:, m, gsl], in_=yt[:])

            xn2g0 = g0p.tile([P, KSUB, 512], F8)
            gu0 = g0p.tile([P, 4 * KSUB, 512], BF16)
            attn_half(0)
            # MLP for the first query half: emitted before the second
            # attention half so the scheduler can fill its exp-bound PE idle
            outproj_half(0)
            ln2_half(0, xn2g0)
            fc1_half(0, xn2g0, gu0)
            attn_half(1)

            pool_attn.release()
            psum_mm2.release()
            psum_ctx.release()
            psum_fc2 = tc.alloc_tile_pool(name=f"psum_fc2{it}", bufs=1, space="PSUM")
            postp = tc.alloc_tile_pool(name=f"postp{it}", bufs=1, side="right")
            xn2g1 = postp.tile([P, KSUB, 512], F8)
            gu1 = postp.tile([P, 4 * KSUB, 512], BF16)
            w2_hi = postp.tile([P, 2 * KSUB, D], BF16)
            for j in range(0, 2 * KSUB, 3):
                nc.sync.dma_start(out=w2_hi[:, j : j + 3], in_=w2d[:, 2 * KSUB + j : 2 * KSUB + j + 3])
            outproj_half(1)
            ln2_half(1, xn2g1)
            # fc2 of the first half hides the LN2/fc1 serial chain of the second
            fc2_half(0, gu0, w2_hi, psum_fc2)
            fc1_half(1, xn2g1, gu1)
            fc2_half(1, gu1, w2_hi, psum_fc2)

            psum_fc2.release()
            psum_mm3.release()
            postp.release()
            g0p.release()
            ctxp.release()
            wop.release()
            w1s.release()
            ptpool.release()
            h1pool.release()
            stage.release()
            w2p.release()
            xbp.release()
            lnrows.release()
            lnp.release()
            persist.release()
    return nc


_NC = None


def _get_nc():
    global _NC
    if _NC is None:
        _NC = build_nc()
    return _NC


def _feature_major(a2d):
    """[T, D'] -> [128, D'//128, T] with feature d at (d%128, d//128)."""
    t, d = a2d.shape
    return np.ascontiguousarray(a2d.T.reshape(d // P, P, t).transpose(1, 0, 2))


def _col_pack(vec):
    """[D'] -> [128, D'//128] with element d at (d%128, d//128)."""
    return np.ascontiguousarray(vec.reshape(-1, P).T)


def _prep_inputs(inputs):
    x = np.asarray(inputs["x"], np.float32)
    ln1_g = np.asarray(inputs["ln1_g"], np.float32)
    ln1_b = np.asarray(inputs["ln1_b"], np.float32)
    W_qkv = np.asarray(inputs["W_qkv"], np.float32)
    b_qkv = np.asarray(inputs["b_qkv"], np.float32)
    W_o = np.asarray(inputs["W_o"], np.float32)
    b_o = np.asarray(inputs["b_o"], np.float32)
    ln2_g = np.asarray(inputs["ln2_g"], np.float32)
    ln2_b = np.asarray(inputs["ln2_b"], np.float32)
    W1 = np.asarray(inputs["W1"], np.float32)
    b1 = np.asarray(inputs["b1"], np.float32)
    W2 = np.asarray(inputs["W2"], np.float32)
    b2 = np.asarray(inputs["b2"], np.float32)

    bf = ml_dtypes.bfloat16
    wqkv_g = (ln1_g[:, None] * W_qkv).astype(bf)

    def _wpack(w2d_):
        return np.ascontiguousarray(w2d_.reshape(KSUB, P, -1).transpose(1, 0, 2))

    f8 = ml_dtypes.float8_e4m3

    def _wpack8(sec):
        return np.ascontiguousarray(
            (W8SCALE * sec).astype(f8).reshape(3, 2, P, D).transpose(2, 0, 1, 3)
        )

    wqkv_gf = ln1_g[:, None] * W_qkv
    wq_p = _wpack8(wqkv_gf[:, 0:D])
    wk_p = _wpack8(wqkv_gf[:, D : 2 * D])
    wv_p = _wpack8(wqkv_gf[:, 2 * D : 3 * D])
    bqkv_f = b_qkv + ln1_b @ W_qkv            # folded LN1 shift
    bqk_p = _col_pack(bqkv_f[: 2 * D].astype(np.float32))   # [128, 12]
    b_v = bqkv_f[2 * D :]
    bo_f = b_o + b_v @ W_o                    # V bias folded into out-proj
    bo_p = _col_pack(bo_f.astype(np.float32))
    wo_p = _wpack8(W_o)
    w1_g = ln2_g[:, None] * W1
    # fp8 DoubleRow layout: [p, m, jj, o, c] = 16*w1_g[256jj+128o+p, 128m+c]
    w1_m = np.ascontiguousarray(
        (W8SCALE * w1_g)
        .astype(f8)
        .reshape(3, 2, P, 4 * KSUB, P)
        .transpose(2, 3, 0, 1, 4)
    )
    b1_f = b1 + ln2_b @ W1
    b1_p = _col_pack(b1_f.astype(np.float32))
    w2_p = np.ascontiguousarray(
        W2.astype(bf).reshape(4 * KSUB, P, D).transpose(1, 0, 2)
    )
    b2_p = _col_pack(b2.astype(np.float32))

    # x16 so normalized fp8 ctx lands in the normal range; out-proj divides back
    sel2_np = np.zeros((2, P), np.float32)
    sel2_np[0, :HD] = W8SCALE
    sel2_np[1, HD:] = W8SCALE
    in_maps = []
    for core in range(N_CORES):
        b, h = divmod(core, 2)
        perm = block_perm(h)
        own = own_blocks(h)
        pos_block = [4 * (pos // 4) + perm[pos % 4] for pos in range(NB)]
        ctx_tok = np.concatenate(
            [np.arange(P * blk, P * (blk + 1)) for blk in pos_block]
        )
        own_tok = np.concatenate([np.arange(P * g, P * (g + 1)) for g in own])
        xcf = _feature_major(x[b][ctx_tok]).astype(ml_dtypes.float8_e4m3)
        xc_np = np.ascontiguousarray(
            xcf.reshape(P, KSUB, NCH, 512).transpose(2, 0, 1, 3)
        )
        xqf = _feature_major(x[b][own_tok]).astype(bf)      # [P, KSUB, SQ]
        xq_np = np.ascontiguousarray(
            xqf.reshape(P, KSUB, 2, 512).transpose(2, 0, 1, 3)
        )
        mask = np.zeros((P, NB, P), np.float32)
        for pos in range(NB):
            krange = P * pos_block[pos] + np.arange(P)
            qrange = P * own[pos // 2] + np.arange(P)
            mask[:, pos, :] = (krange[:, None] <= qrange[None, :])
        in_maps.append(
            {
                "xc": xc_np,
                "xq": xq_np,
                "wk": wk_p,
                "wv": wv_p,
                "wq": wq_p,
                "wo": wo_p,
                "w1": w1_m,
                "w2": w2_p,
                "bqk": bqk_p,
                "bo": bo_p,
                "b1": b1_p,
                "b2": b2_p,
                "mask": mask.astype(ml_dtypes.float8_e4m3),
                "sel2": sel2_np.astype(bf),
            }
        )
    return in_maps


def _assemble(results):
    y = np.empty((B, S, D), np.float32)
    for c in range(N_CORES):
        b, h = divmod(c, 2)
        blocks = own_blocks(h)
        yt = results[c]["yT"]  # [128, 6, 1024]
        for j, g in enumerate(blocks):
            chunk = yt[:, :, P * j : P * (j + 1)]          # [128, 6, 128]
            y[b, P * g : P * (g + 1), :] = (
                chunk.transpose(1, 0, 2).reshape(D, P).T
            )
    return y


def _run(inputs, trace=False):
    nc = _get_nc()
    in_maps = _prep_inputs(inputs)
    res = run_bass_kernel_spmd(nc, in_maps, list(range(N_CORES)), trace=trace)
    return _assemble(res.results), res


def kernel(**inputs):
    out, _ = _run(inputs)
    return out

